# revision 1
# baseline (speedup 1.0000x reference)
"""Multi-head self-attention (B=4, T=2048, E=1024, H=16, Dh=64) on 8 trn2 cores.

Sharding: core c handles batch b=c//2, query half hf=c%2 (1024 query tokens),
with all 2048 keys/values of its batch (KV projection duplicated across the
2 cores sharing a batch). No collectives. Host pre-transposes x so every
matmul on-device consumes natural layouts.

Per-core math (tokens rotated so the core's 1024 query tokens come first;
softmax over keys is invariant to key-order permutation):
  xT        [E=1024, T=2048]   (input, pre-transposed on host)
  qT_h      [64, 1024] = Wq_h.T @ xT[:, :1024] + bq_h   (bias folded; k-bias
  kT_h      [64, 2048] = Wk_h.T @ xT                     is softmax-invariant
  v_h       [2048, 64] = (xT.T @ Wv_h) + bv_h            and dropped)
  eT_h      [2048, 1024] = kT_h.T-rows ... e[k,q] = k_h[k]·q_h[q]
  p = exp(e/8);  out_hT [64,1024] = (v_h.T @ p) / colsum(p)
  out       [1024, 1024] = concat_h(out_hT).T @ fc_w + fc_b
"""

import numpy as np

E = 1024
T = 2048  # keys per core (full batch)
TQ = 1024  # queries per core
H = 16
DH = 64
NG = 4  # head groups
HPG = H // NG  # heads per group = 4
GC = HPG * DH // 128  # 128-row chunks of group qkv cols = 2 (one per head pair)
EC = E // 128  # 8 e-chunks
N_CORES = 8
SCALE = DH ** -0.5

_CACHE = {}


def _build():
    import concourse.bass as bass
    import concourse.mybir as mybir
    import concourse.tile as tile
    import concourse.bass_isa as bass_isa
    from concourse import bacc
    from contextlib import ExitStack

    f32 = mybir.dt.float32
    f32r = mybir.dt.float32r
    AF = mybir.ActivationFunctionType
    OP = mybir.AluOpType

    nc = bacc.Bacc("TRN2", target_bir_lowering=False, debug=False)

    xt = nc.declare_dram_parameter("xt", [E, T], f32r, isOutput=False).ap()
    qkv_w = nc.declare_dram_parameter("qkv_w", [E, 3 * E], f32r, isOutput=False).ap()
    q_bias = nc.declare_dram_parameter("q_bias", [E, 1], f32, isOutput=False).ap()
    v_bias = nc.declare_dram_parameter("v_bias", [128, E], f32, isOutput=False).ap()
    fc_w = nc.declare_dram_parameter("fc_w", [E, E], f32r, isOutput=False).ap()
    fc_b = nc.declare_dram_parameter("fc_b", [128, E], f32, isOutput=False).ap()
    out = nc.declare_dram_parameter("out", [TQ, E], f32, isOutput=True).ap()
    dbg = _CACHE.get("debug", False)
    if dbg:
        qt_dbg = nc.declare_dram_parameter("qt_dbg", [NG, 128, GC, TQ], f32, isOutput=True).ap()
        kt_dbg = nc.declare_dram_parameter("kt_dbg", [NG, 128, GC, T], f32, isOutput=True).ap()
        va_dbg = nc.declare_dram_parameter("va_dbg", [NG, 128, T // 128, GC * 192], f32, isOutput=True).ap()
        ot_dbg = nc.declare_dram_parameter("ot_dbg", [128, H * DH // 128, TQ], f32, isOutput=True).ap()
        ps_dbg = nc.declare_dram_parameter("ps_dbg", [2, 128, 512], f32, isOutput=True).ap()
        rc_dbg = nc.declare_dram_parameter("rc_dbg", [128, 512], f32, isOutput=True).ap()

    # [p, ec, cols] views with e on partitions
    xt_r = xt.rearrange("(c p) t -> p c t", p=128)
    qkv_w_r = qkv_w.rearrange("(c p) n -> p c n", p=128)
    q_bias_r = q_bias.rearrange("(c p) one -> p c one", p=128)
    fc_w_r = fc_w.rearrange("(c p) n -> p c n", p=128)

    def rd(ap):  # f32 view for non-matmul reads of f32r data
        return ap.bitcast(f32)

    with tile.TileContext(nc) as tc, ExitStack() as ctx:
        pool_const = ctx.enter_context(tc.tile_pool(name="const", bufs=1))
        pool_outT = ctx.enter_context(tc.tile_pool(name="outT", bufs=1))
        # attn output, transposed: [hd-chunk(pair P), q]
        outT = pool_outT.tile([128, H * DH // 128, TQ], f32r)

        qb_sb = pool_const.tile([128, EC, 1], f32)
        nc.sync.dma_start(out=qb_sb, in_=q_bias_r)
        vb_sb = pool_const.tile([128, E], f32)
        nc.sync.dma_start(out=vb_sb, in_=v_bias)

        # mask for denominator broadcast: psum_r = mask.T @ dsb puts
        # dsb row 64 (even denom) on rows 0:64 and dsb row 0 (odd denom)
        # on rows 64:128
        dmask = pool_const.tile([128, 128], f32r)
        nc.vector.tensor_scalar(dmask, vb_sb[:, 0:128], 0.0, 0.0, OP.mult, OP.add)
        nc.vector.tensor_scalar(
            dmask[64:65, 0:64], vb_sb[64:65, 0:64], 0.0, 1.0, OP.mult, OP.add)
        nc.vector.tensor_scalar(
            dmask[0:1, 64:128], vb_sb[0:1, 0:64], 0.0, 1.0, OP.mult, OP.add)

        for rep in range(_CACHE.get("reps", 1)):
            with ExitStack() as actx:
                pool_xt = actx.enter_context(tc.tile_pool(name="xt", bufs=2))
                pool_w = actx.enter_context(tc.tile_pool(name="w", bufs=1))
                pool_kqv = actx.enter_context(tc.tile_pool(name="kqv", bufs=1))
                pool_exp = actx.enter_context(tc.tile_pool(name="exp", bufs=10))
                pool_rec = actx.enter_context(tc.tile_pool(name="rec", bufs=2))
                ps_mm = actx.enter_context(tc.tile_pool(name="psmm", bufs=2, space="PSUM"))
                ps_e = actx.enter_context(tc.tile_pool(name="pse", bufs=2, space="PSUM"))
                ps_o = actx.enter_context(tc.tile_pool(name="pso", bufs=2, space="PSUM"))

                for g in range(NG):
                    gw = HPG * DH  # 256 qkv cols per group
                    # ---- group weights (resident) ----
                    wq_g = pool_w.tile([128, EC, gw], f32r, tag="wq")
                    nc.sync.dma_start(out=wq_g, in_=qkv_w_r[:, :, g * gw:(g + 1) * gw])
                    wk_g = pool_w.tile([128, EC, gw], f32r, tag="wk")
                    nc.sync.dma_start(out=wk_g, in_=qkv_w_r[:, :, E + g * gw:E + (g + 1) * gw])
                    wv_g = pool_w.tile([128, EC, gw], f32r, tag="wv")
                    nc.sync.dma_start(out=wv_g, in_=qkv_w_r[:, :, 2 * E + g * gw:2 * E + (g + 1) * gw])

                    kT_g = pool_kqv.tile([128, HPG, T], f32r, tag="kT")
                    qT_g = pool_kqv.tile([128, HPG, TQ], f32r, tag="qT")
                    # zero the unused partition half of each head's slab so QK can
                    # run as a standard K=128 matmul (data half + zero half)
                    for j in range(HPG):
                        zlo, zhi = (64, 128) if j % 2 == 0 else (0, 64)
                        for half in range(T // 1024):
                            nc.vector.tensor_scalar(
                                kT_g[zlo:zhi, j, half * 1024:(half + 1) * 1024],
                                vb_sb[zlo:zhi, :], 0.0, 0.0, OP.mult, OP.add)
                        nc.vector.tensor_scalar(
                            qT_g[zlo:zhi, j, :], vb_sb[zlo:zhi, :], 0.0, 0.0,
                            OP.mult, OP.add)
                    v_g = pool_kqv.tile([128, T // 128, GC * 192], f32r, tag="v")
                    va4 = v_g.rearrange("p t (pr c) -> p t pr c", pr=GC)
                    # ones column at block col 64 (denominator weights); cols
                    # 65:128 are never-read garbage (their psum rows are unused)
                    ones_src = vb_sb[:, 0:T // 128 * GC].rearrange(
                        "p (t pr one) -> p t pr one", t=T // 128, pr=GC)
                    nc.vector.tensor_scalar(
                        va4[:, :, :, 64:65], ones_src, 0.0, 1.0, OP.mult, OP.add)
                    z1 = vb_sb[:, 0:1024].rearrange("p (t pr c) -> p t pr c", t=T // 128, pr=GC)
                    nc.vector.tensor_scalar(
                        va4[:, :, :, 65:97], z1, 0.0, 0.0, OP.mult, OP.add)
                    z2 = vb_sb[:, 0:992].rearrange("p (t pr c) -> p t pr c", t=T // 128, pr=GC)
                    nc.vector.tensor_scalar(
                        va4[:, :, :, 97:128], z2, 0.0, 0.0, OP.mult, OP.add)

                    # ---- phase A: projections for this group ----
                    for tb in range(T // 512):
                        xt_tb = pool_xt.tile([128, EC, 512], f32r, tag="xt")
                        nc.sync.dma_start(out=xt_tb, in_=xt_r[:, :, tb * 512:(tb + 1) * 512])

                        for ch in range(GC):  # kT rows chunk (= local pair)
                            psA = ps_mm.tile([128, 512], f32, tag="mm")
                            for ec in range(EC):
                                nc.tensor.matmul(
                                    psA,
                                    wk_g[:, ec, ch * 128:(ch + 1) * 128],
                                    xt_tb[:, ec, :],
                                    start=(ec == 0), stop=(ec == EC - 1),
                                )
                            nc.vector.tensor_copy(
                                kT_g[0:64, 2 * ch, tb * 512:(tb + 1) * 512], psA[0:64, :])
                            nc.vector.tensor_copy(
                                kT_g[64:128, 2 * ch + 1, tb * 512:(tb + 1) * 512], psA[64:128, :])

                        for tc_ in range(4):  # token chunks within tb
                            psV = ps_mm.tile([128, gw], f32, tag="mm")
                            for ec in range(EC):
                                nc.tensor.matmul(
                                    psV,
                                    xt_tb[:, ec, tc_ * 128:(tc_ + 1) * 128],
                                    wv_g[:, ec, :],
                                    start=(ec == 0), stop=(ec == EC - 1),
                                )
                            psv4 = psV.rearrange("p (pr h d) -> p pr h d", pr=GC, h=2)
                            vb4 = vb_sb[:, g * gw:(g + 1) * gw].rearrange(
                                "p (pr h d) -> p pr h d", pr=GC, h=2)
                            K0 = tb * 4 + tc_
                            nc.vector.tensor_add(
                                va4[:, K0, :, 0:64], psv4[:, :, 0, :], vb4[:, :, 0, :])
                            nc.vector.tensor_add(
                                va4[:, K0, :, 128:192], psv4[:, :, 1, :], vb4[:, :, 1, :])

                        if tb < TQ // 512:  # queries live in token cols 0..1024
                            for ch in range(GC):
                                P = g * GC + ch
                                psQ = ps_mm.tile([128, 512], f32, tag="mm")
                                for ec in range(EC):
                                    nc.tensor.matmul(
                                        psQ,
                                        wq_g[:, ec, ch * 128:(ch + 1) * 128],
                                        xt_tb[:, ec, :],
                                        start=(ec == 0), stop=(ec == EC - 1),
                                    )
                                nc.vector.tensor_scalar(
                                    qT_g[0:64, 2 * ch, tb * 512:(tb + 1) * 512],
                                    psQ[0:64, :], qb_sb[0:64, P, :], None, OP.add,
                                )
                                nc.vector.tensor_scalar(
                                    qT_g[64:128, 2 * ch + 1, tb * 512:(tb + 1) * 512],
                                    psQ[64:128, :], qb_sb[64:128, P, :], None, OP.add,
                                )

                    if dbg:
                        nc.sync.dma_start(out=qt_dbg[g], in_=rd(qT_g))
                        nc.sync.dma_start(out=kt_dbg[g], in_=rd(kT_g))
                        nc.sync.dma_start(out=va_dbg[g], in_=rd(v_g))

                    # ---- phase B: attention for this group ----
                    for p in range(GC):  # local head pair; heads at part 0:64, 64:128
                        for qb in range(TQ // 512):
                            P = g * GC + p
                            psO_e = ps_o.tile([128, 512], f32, tag="po_e")
                            psO_o = ps_o.tile([128, 512], f32, tag="po_o")
                            for kb in range(4):  # blocks of 4 k-chunks
                                exps = {}
                                for kc in range(4):
                                    K = kb * 4 + kc
                                    for hf in range(2):
                                        j = p * 2 + hf
                                        psE = ps_e.tile([128, 512], f32, tag="pe")
                                        nc.tensor.matmul(
                                            psE,
                                            kT_g[:, j, K * 128:(K + 1) * 128],
                                            qT_g[:, j, qb * 512:(qb + 1) * 512],
                                            start=True, stop=True,
                                        )
                                        ex = pool_exp.tile([128, 512], f32r, tag="ex")
                                        nc.scalar.activation(ex, psE, AF.Exp, scale=SCALE)
                                        exps[(kc, hf)] = ex
                                for kc in range(4):
                                    K = kb * 4 + kc
                                    blk = va4[:, K, p, :]  # [128, 192]
                                    # even head: AV rows 0:64, denom row 64
                                    nc.tensor.matmul(
                                        psO_e, blk[:, 0:128], exps[(kc, 0)],
                                        start=(K == 0), stop=(K == T // 128 - 1),
                                    )
                                    # odd head: denom row 0, AV rows 64:128
                                    nc.tensor.matmul(
                                        psO_o, blk[:, 64:192], exps[(kc, 1)],
                                        start=(K == 0), stop=(K == T // 128 - 1),
                                    )
                            # denominator rows -> SBUF, broadcast via mask
                            # matmul, then reciprocal (all standard-mode, aligned)
                            dsb = pool_rec.tile([128, 512], f32r, tag="dsb")
                            nc.vector.tensor_scalar(
                                dsb, vb_sb[:, 0:512], 0.0, 0.0, OP.mult, OP.add)
                            nc.vector.tensor_copy(dsb[64:65, :], psO_e[64:65, :])
                            nc.vector.tensor_copy(dsb[0:1, :], psO_o[0:1, :])
                            psR = ps_mm.tile([128, 512], f32, tag="mm")
                            nc.tensor.matmul(psR, dmask, dsb, start=True, stop=True)
                            recipb = pool_rec.tile([128, 512], f32, tag="recipb")
                            nc.vector.reciprocal(recipb, psR)
                            qs = slice(qb * 512, (qb + 1) * 512)
                            if dbg and g == 0 and p == 0 and qb == 0:
                                pse_sb = pool_rec.tile([128, 512], f32, tag="dbg_e")
                                nc.vector.tensor_copy(pse_sb, psO_e)
                                nc.sync.dma_start(out=ps_dbg[0], in_=pse_sb)
                                pso_sb = pool_rec.tile([128, 512], f32, tag="dbg_o")
                                nc.vector.tensor_copy(pso_sb, psO_o)
                                nc.sync.dma_start(out=ps_dbg[1], in_=pso_sb)
                                nc.sync.dma_start(out=rc_dbg, in_=recipb)
                            nc.vector.tensor_tensor(
                                outT[0:64, P, qs], psO_e[0:64, :], recipb[0:64, :], OP.mult)
                            nc.vector.tensor_tensor(
                                outT[64:128, P, qs], psO_o[64:128, :], recipb[64:128, :], OP.mult)

            if dbg:
                nc.sync.dma_start(out=ot_dbg, in_=rd(outT))

            # ---- phase C: output projection ----
            with ExitStack() as cctx:
                pool_fc = cctx.enter_context(tc.tile_pool(name="fc", bufs=1))
                pool_oc = cctx.enter_context(tc.tile_pool(name="oc", bufs=3))
                ps_c = cctx.enter_context(tc.tile_pool(name="psc", bufs=2, space="PSUM"))

                fcw_sb = pool_fc.tile([128, EC, E], f32r)
                nc.sync.dma_start(out=fcw_sb, in_=fc_w_r)
                fcb_sb = pool_fc.tile([128, E], f32)
                nc.sync.dma_start(out=fcb_sb, in_=fc_b)

                for qc in range(TQ // 128):
                    ot = pool_oc.tile([128, E], f32, tag="ot")
                    for ob in range(2):
                        psC = ps_c.tile([128, 512], f32, tag="pc")
                        for hc in range(EC):
                            nc.tensor.matmul(
                                psC,
                                outT[:, hc, qc * 128:(qc + 1) * 128],
                                fcw_sb[:, hc, ob * 512:(ob + 1) * 512],
                                start=(hc == 0), stop=(hc == EC - 1),
                            )
                        nc.vector.tensor_add(ot[:, ob * 512:(ob + 1) * 512], psC,
                                             fcb_sb[:, ob * 512:(ob + 1) * 512])
                    nc.sync.dma_start(out=out[qc * 128:(qc + 1) * 128, :], in_=ot)

    nc.compile()
    return nc


def _get_nc():
    if "nc" not in _CACHE:
        _CACHE["nc"] = _build()
    return _CACHE["nc"]


def _in_maps(x, qkv_w, qkv_b, fc_w, fc_b):
    qkv_w = np.ascontiguousarray(qkv_w, dtype=np.float32)
    fc_w = np.ascontiguousarray(fc_w, dtype=np.float32)
    q_b = np.ascontiguousarray(qkv_b[:E].reshape(E, 1), dtype=np.float32)
    v_b = np.ascontiguousarray(
        np.broadcast_to(qkv_b[2 * E:3 * E], (128, E)), dtype=np.float32)
    f_b = np.ascontiguousarray(np.broadcast_to(fc_b, (128, E)), dtype=np.float32)
    maps = []
    for c in range(N_CORES):
        b, hf = c // 2, c % 2
        xb = x[b]
        if hf == 1:
            xb = np.concatenate([xb[TQ:], xb[:TQ]], axis=0)
        xt_c = np.ascontiguousarray(xb.T, dtype=np.float32)
        maps.append({"xt": xt_c, "qkv_w": qkv_w, "q_bias": q_b, "v_bias": v_b,
                     "fc_w": fc_w, "fc_b": f_b})
    return maps


def run(x, qkv_w, qkv_b, fc_w, fc_b, trace=False):
    from concourse.bass_utils import run_bass_kernel_spmd

    nc = _get_nc()
    maps = _in_maps(np.asarray(x), np.asarray(qkv_w), np.asarray(qkv_b),
                    np.asarray(fc_w), np.asarray(fc_b))
    res = run_bass_kernel_spmd(nc, maps, list(range(N_CORES)), trace=trace)
    B = x.shape[0]
    full = np.empty((B, T, E), dtype=np.float32)
    for c in range(N_CORES):
        b, hf = c // 2, c % 2
        full[b, hf * TQ:(hf + 1) * TQ] = res.results[c]["out"]
    return full, res


def kernel(x, qkv_w, qkv_b, fc_w, fc_b):
    full, _ = run(x, qkv_w, qkv_b, fc_w, fc_b, trace=False)
    return full

